# revision 5
# baseline (speedup 1.0000x reference)
"""CenterNet NMS-decode kernel for 8 Trainium2 NeuronCores.

Strategy (pure data parallel, 4 images/core):
  - Device (Bass/Tile): channel-max over the 80 heatmap channels — the
    memory-bound bulk (160 MiB streamed, 1 MiB out). Layout packs all 4
    images into the 128 partitions (partition = img*32 + h//4), so every
    DMA descriptor is a contiguous 2 KiB run. All input DMAs are issued
    up-front (the whole 160 KiB/partition shard is SBUF-resident), so the
    two HWDGE rings stream at the HBM roofline with no compute coupling.
    The vector engine chases the stream with in-place pairwise-max trees;
    chunk sizes taper (16...1 channels) so the post-stream tail is tiny.
  - Host: sigmoid, 3x3 peak keep, per-image top-k, and the batch-0-index
    gather of boxes/labels. All host ops are either exact max/compare ops
    or exact f32 arithmetic (x*16 is an exponent shift; the final subtract
    is a single IEEE rounding), so results match the jax reference bitwise
    except for sigmoid ULPs, which cancel in the order-based peak mask.
"""

import os
import sys

import numpy as np

for _p in ("/opt/trn_rl_repo",):
    if os.path.isdir(_p) and _p not in sys.path:
        sys.path.insert(0, _p)

B, C, H, W = 32, 80, 128, 128
N_CORES = 8
IPC = B // N_CORES  # images per core
K_TOP = 100

# channel chunk sizes (decreasing tail) and ring assignment (0=sync, 1=scalar)
CHUNKS = [16, 16, 16, 8, 8, 8, 4, 2, 1, 1]
RINGS = [0, 1, 0, 1, 0, 1, 1, 1, 0, 1]

_CACHE = {}


def _emit_body(nc, tc, cin, hm, heat, tag_prefix=""):
    import concourse.tile as tile
    from concourse import mybir

    rings = [nc.sync, nc.scalar]
    tiles = []
    c0 = 0
    for k, cc in enumerate(CHUNKS):
        ct = cin.tile([128, cc * 512], mybir.dt.float32, tag=f"{tag_prefix}c{k}")
        # per-image DMAs: images 0,1 live on partitions 0-63 (SDMA engines
        # 0-7) and ride the sync ring; images 2,3 on partitions 64-127
        # (engines 8-15) ride scalar — the rings never contend for engines.
        for i in range(IPC):
            src = hm[i, c0 : c0 + cc].rearrange("c (q h4) w -> q c (h4 w)", h4=4)
            rings[i // 2].dma_start(out=ct[32 * i : 32 * (i + 1), :], in_=src)
        tiles.append(ct)
        c0 += cc
    # vector engine: in-place pairwise-max tree per chunk, then fold into acc
    acc = None
    for k, cc in enumerate(CHUNKS):
        s = tiles[k]
        n = cc
        while n > 1:
            lo = n // 2
            hi = n - lo  # hi >= lo
            # fold the low half into the tail of the high half
            nc.vector.tensor_max(
                s[:, : lo * 512], s[:, : lo * 512], s[:, hi * 512 : n * 512]
            )
            n = hi
        if acc is None:
            acc = s
        else:
            nc.vector.tensor_max(acc[:, :512], acc[:, :512], s[:, :512])
    dst = heat.rearrange("i (q h4) w -> (i q) (h4 w)", h4=4)
    nc.sync.dma_start(out=dst, in_=acc[:, :512])


def _build(reps=1):
    import concourse.tile as tile
    from concourse import bacc, mybir

    nc = bacc.Bacc(
        "TRN2",
        target_bir_lowering=False,
        debug=False,
        enable_asserts=False,
        num_devices=N_CORES,
    )
    hm = nc.dram_tensor("hm", [IPC, C, H, W], mybir.dt.float32, kind="ExternalInput").ap()
    heat = nc.dram_tensor("heat", [IPC, H, W], mybir.dt.float32, kind="ExternalOutput").ap()

    with tile.TileContext(nc) as tc:
        with tc.tile_pool(name="cin", bufs=1) as cin:
            for _r in range(reps):
                _emit_body(nc, tc, cin, hm, heat)
    nc.compile()
    return nc


def _get_nc(reps=1):
    key = ("nc", reps)
    if key not in _CACHE:
        _CACHE[key] = _build(reps)
    return _CACHE[key]


def _build_loop(iters):
    """Same kernel body wrapped in a hardware For_i loop. Used only by
    test.py for noise-immune differential timing (device time >> host
    jitter); kernel() itself uses the single-shot _build(1) program."""
    import concourse.tile as tile
    from concourse import bacc, mybir

    nc = bacc.Bacc(
        "TRN2",
        target_bir_lowering=False,
        debug=False,
        enable_asserts=False,
        num_devices=N_CORES,
    )
    hm = nc.dram_tensor("hm", [IPC, C, H, W], mybir.dt.float32, kind="ExternalInput").ap()
    heat = nc.dram_tensor("heat", [IPC, H, W], mybir.dt.float32, kind="ExternalOutput").ap()

    with tile.TileContext(nc) as tc:
        with tc.tile_pool(name="cin", bufs=1) as cin:
            with tc.For_i(0, iters, 1) as _i:
                _emit_body(nc, tc, cin, hm, heat)
    nc.compile()
    return nc


def _run_device(heatmap, trace=False, reps=1, **kw):
    from concourse.bass_utils import run_bass_kernel_spmd

    nc = _get_nc(reps)
    in_maps = [
        {"hm": np.ascontiguousarray(heatmap[IPC * i : IPC * (i + 1)])}
        for i in range(N_CORES)
    ]
    res = run_bass_kernel_spmd(nc, in_maps, list(range(N_CORES)), trace=trace, **kw)
    heat = np.concatenate([res.results[i]["heat"] for i in range(N_CORES)], axis=0)
    return heat, res


def _sigmoid(x):
    # Default jax backend, matching wherever reference() would run: the
    # score column must be bitwise-identical to the reference's sigmoid.
    import jax
    import jax.numpy as jnp

    return np.asarray(jax.nn.sigmoid(jnp.asarray(x)))


def _maxpool3(m):
    # 3x3 stride-1 SAME max pool over the last two axes, exact shifted maxes.
    hh = m.copy()
    hh[:, :, :-1] = np.maximum(hh[:, :, :-1], m[:, :, 1:])
    hh[:, :, 1:] = np.maximum(hh[:, :, 1:], m[:, :, :-1])
    vv = hh.copy()
    vv[:, :-1] = np.maximum(vv[:, :-1], hh[:, 1:])
    vv[:, 1:] = np.maximum(vv[:, 1:], hh[:, :-1])
    return vv


def _postprocess(heat, heatmap, wh):
    scores = _sigmoid(heat)  # [B,H,W]
    keep = scores == _maxpool3(scores)
    score_map = (scores * keep).reshape(B, -1)

    idx = np.argsort(-score_map, axis=1, kind="stable")[:, :K_TOP]
    top_score = np.take_along_axis(score_map, idx, axis=1)
    idx0 = idx[0]

    px = (idx0 % W).astype(np.float32) * np.float32(4.0)
    py = (idx0 // W).astype(np.float32) * np.float32(4.0)
    wh_g = wh.reshape(B, 4, H * W)[:, :, idx0] * np.float32(16.0)  # exact
    x1 = px[None] - wh_g[:, 0]
    y1 = py[None] - wh_g[:, 1]
    x2 = px[None] + wh_g[:, 2]
    y2 = py[None] + wh_g[:, 3]
    labels = np.argmax(heatmap.reshape(B, C, H * W)[:, :, idx0], axis=1)
    out = np.stack(
        [x1, y1, x2, y2, top_score, labels.astype(np.float32)], axis=2
    ).astype(np.float32)
    return out


def kernel(heatmap, wh):
    heatmap = np.ascontiguousarray(np.asarray(heatmap, dtype=np.float32))
    wh = np.ascontiguousarray(np.asarray(wh, dtype=np.float32))
    heat, _ = _run_device(heatmap)
    return _postprocess(heat, heatmap, wh)
